# revision 10
# baseline (speedup 1.0000x reference)
"""DiT block kernel for 8 trn2 NeuronCores — fp8 DoubleRow version.

Sharding: core c -> (batch b=c//2, query-token half h=c%2). Each core
computes the full block for its 512 query tokens (K/V compute for all
1024 tokens of its batch is replicated within the pair) -> zero
collectives. Activations are feature-major ([D on partitions, tokens on
free]); weights are used in natural [in, out] layout as matmul lhsT.

All large matmuls run in fp8 e4m3 with MatmulPerfMode.DoubleRow (two
128-deep K slices per instruction, 2x PE throughput, half the weight
DMA). Weights are pre-scaled on host (x32 for qkv so the stored q/k/v
fit e4m3 range, x64 elsewhere) so values sit in e4m3's normal range;
corrections fold into the exp scale (1/1024), the softmax rowsum ones
constant (32.0) and the residual gates (1/64) — q/k/v epilogues are
pure psum->fp8 copies. Biases enter psum via K=1 fp8 DoubleRow matmuls
([bias*scale; 0] pair rows against an all-ones moving row).

qkv and attention are interleaved (qk(g), scores(g) emitted per head
pair, v blocks before the first av) so the PE stays busy while ACT
streams the 64 softmax exps. LN stats/softmax normalization/residuals
stay fp32/f16; psum reads go to DVE, f16 SBUF work to Pool(gpsimd),
exp/gelu/tanh/sqrt to ACT.
"""
import numpy as np

import concourse.bass as bass
import concourse.tile as tile
import concourse.mybir as mybir
from concourse.bass_utils import run_bass_kernel_spmd
from concourse.vector_clock import ScopedClock
from concourse.alu_op_type import AluOpType

dt = mybir.dt
AF = mybir.ActivationFunctionType
PM = mybir.MatmulPerfMode

P = 128
B, NT, D, H = 4, 1024, 1024, 16
DH = D // H            # 64
DFF = 4 * D            # 4096
KC = D // P            # 8
KP = KC // 2           # 4 DoubleRow K-pairs
LT = NT // 2           # 512 local query tokens
GATE = 0.1
EPS = 1e-5
EXP_BIAS = -3.0        # constant shift inside exp; cancels in softmax
WQ = 32.0              # host pre-scale for qkv weights (q/k/v stored x32)
WS = 64.0              # host pre-scale for the other weights
INV = 1.0 / WS
EXP_SCALE = DH ** -0.5 / (WQ * WQ)   # scores psum holds (32q)·(32k)
RS_ONES = WQ           # rowsum ones constant: recip absorbs v's x32


class SplitDrainTileContext(tile.TileContext):
    """Tail drain in this walrus build holds few sync waits; spill the
    rest onto chained SP nops (runs before the sem-clear barrier, so
    semantics are preserved)."""

    MAX_TAIL_WAITS = 1

    def _drain_and_barrier(self, tick_clock, wait_clock):
        drain_inst = self.nc.sync.drain()
        wait_clock.add_sem_waits(
            drain_inst.ins, ScopedClock({None: tick_clock.global_clock})
        )
        si = drain_inst.ins.sync_info
        waits = list(si.on_wait) if si else []
        if len(waits) > self.MAX_TAIL_WAITS:
            drain_inst.ins.sync_info = mybir.SyncInfo(
                on_wait=waits[: self.MAX_TAIL_WAITS],
                on_update=list(si.on_update) if si else [],
            )
            rest = waits[self.MAX_TAIL_WAITS:]
            for i in range(0, len(rest), self.MAX_TAIL_WAITS):
                nop = self.nc.sync.nop()
                nop.ins.sync_info = mybir.SyncInfo(
                    on_wait=rest[i : i + self.MAX_TAIL_WAITS], on_update=[]
                )
        self.nc.all_engine_barrier()
        assert self.sems is not None
        popped = self.nc._tile_sem_poison_stack.pop()
        assert popped is self._sem_poison
        self.nc.clear_and_free_semaphores(list(self.sems.allocated().values()))
        self.nc.all_engine_barrier()


def _legalize_waits(nc, max_waits=1):
    """This walrus build accepts at most one sync wait per instruction.
    Move surplus waits onto same-engine NoOps inserted just before the
    offending instruction (engine FIFO order preserves semantics)."""
    fix = 0
    for bb in nc.main_func.blocks:
        insts = list(bb.instructions)
        out = []
        for inst in insts:
            si = inst.sync_info
            waits = list(si.on_wait) if si else []
            if len(waits) > max_waits:
                keep = waits[-max_waits:]
                for w in waits[:-max_waits]:
                    nop = mybir.InstNoOp(name=f"I-wfix{fix}")
                    fix += 1
                    nop.engine = inst.engine
                    nop.sync_info = mybir.SyncInfo(on_wait=[w], on_update=[])
                    out.append(nop)
                inst.sync_info = mybir.SyncInfo(
                    on_wait=keep, on_update=list(si.on_update) if si else [])
            out.append(inst)
        if len(out) != len(insts):
            bb.instructions = out
    return fix


def _build(legalize=True):
    nc = bass.Bass(target_bir_lowering=False, debug=False,
                   dynamic_dma_scratch_size=2048)
    f32, f16, f8 = dt.float32, dt.float16, dt.float8e4

    xt = nc.dram_tensor("xt", [D, NT], f16, kind="ExternalInput")
    cond8 = nc.dram_tensor("cond8", [P, KC, 1], f8, kind="ExternalInput")
    qkvw = nc.dram_tensor("qkvw", [D, 3 * D], f8, kind="ExternalInput")
    qkb2 = nc.dram_tensor("qkb2", [1, 2, 2 * D], f8, kind="ExternalInput")
    bv2 = nc.dram_tensor("bv2", [1, 2, D], f8, kind="ExternalInput")
    projw = nc.dram_tensor("projw", [D, D], f8, kind="ExternalInput")
    pb2 = nc.dram_tensor("pb2", [1, 2, D], f8, kind="ExternalInput")
    fc1w = nc.dram_tensor("fc1w", [D, DFF], f8, kind="ExternalInput")
    f1b2 = nc.dram_tensor("f1b2", [1, 2, DFF], f8, kind="ExternalInput")
    fc2w = nc.dram_tensor("fc2w", [D, DFF], f8, kind="ExternalInput")  # host-rearranged
    f2b2 = nc.dram_tensor("f2b2", [1, 2, D], f8, kind="ExternalInput")
    modw = nc.dram_tensor("modw", [12 * P, 4 * D], f8, kind="ExternalInput")  # host-rearranged
    modbf = nc.dram_tensor("modbf", [P, 6 * KC], f32, kind="ExternalInput")
    lnf = nc.dram_tensor("lnf", [P, 4 * KC], f32, kind="ExternalInput")
    outt = nc.dram_tensor("outt", [D, LT], f32, kind="ExternalOutput")

    with SplitDrainTileContext(nc) as tc:
        with tc.tile_pool(name="cp", bufs=1) as cp, \
             tc.tile_pool(name="ar", bufs=1) as ar, \
             tc.tile_pool(name="rot", bufs=4) as rot, \
             tc.tile_pool(name="psA", bufs=4, space="PSUM") as psA, \
             tc.tile_pool(name="psB", bufs=2, space="PSUM") as psB:

            def pp():    # [P, 512] f32 psum, 4 rotating banks
                return psA.tile([P, 512], f32, tag="pp", name="pp")

            def pbig():  # [P, 1024] f32 psum, 2 rotating 2-bank tiles
                return psB.tile([P, 1024], f32, tag="big", name="big")

            ones16 = cp.tile([P, P], f16, tag="ones16")
            nc.vector.memset(ones16[:], 1.0)
            rso = cp.tile([P, 2, P], f8, tag="rso")
            nc.vector.memset(rso[:], RS_ONES)
            o2row = cp.tile([1, 2, LT], f8, tag="o2row")
            nc.vector.memset(o2row[:], 1.0)
            ones_lhs = cp.tile([1, 2, D], f8, tag="ones_lhs")
            nc.vector.memset(ones_lhs[:], 1.0)
            expb = cp.tile([P, 1], f32, tag="expb")
            nc.vector.memset(expb[:], EXP_BIAS)

            # ---- resident small inputs ----
            condt = cp.tile([P, KC, 1], f8, tag="condt")
            nc.sync.dma_start(condt[:], cond8[:])
            lnt = cp.tile([P, 4, KC], f32, tag="lnt")
            nc.sync.dma_start(lnt[:], lnf.rearrange("p (w c) -> p w c", c=KC))
            modbt = cp.tile([P, 6, KC], f32, tag="modbt")
            nc.sync.dma_start(modbt[:], modbf.rearrange("p (w c) -> p w c", c=KC))
            qkbt = cp.tile([1, 2, 2 * D], f8, tag="qkbt")
            nc.sync.dma_start(qkbt[:], qkb2[:])
            bvt = cp.tile([1, 2, D], f8, tag="bvt")
            nc.sync.dma_start(bvt[:], bv2[:])
            pbt = cp.tile([1, 2, D], f8, tag="pbt")
            nc.sync.dma_start(pbt[:], pb2[:])
            f1bt = cp.tile([1, 2, DFF], f8, tag="f1bt")
            nc.sync.dma_start(f1bt[:], f1b2[:])
            f2bt = cp.tile([1, 2, D], f8, tag="f2bt")
            nc.sync.dma_start(f2bt[:], f2b2[:])

            # x, feature-major f16, 2 MiB (tag A4 later reused by h16)
            xf = ar.tile([P, KC, NT], f16, tag="A4")
            nc.sync.dma_start(xf[:], xt.rearrange("(c p) t -> p c t", p=P))

            # ---- S1: modulation matvecs (feature-major out) ----
            modv = cp.tile([P, 6, KC], f32, tag="modv")

            def mod_matvec(w):
                pm = pp()  # [P, 8] accum lives in a [P,512] slot
                for half in range(2):
                    mwt = ar.tile([P, KC, 512], f8, tag="Q1", bufs=3)
                    nc.sync.dma_start(
                        mwt[:],
                        modw[(w * 2 + half) * P:(w * 2 + half + 1) * P, :]
                        .rearrange("p (c m) -> p c m", m=512))
                    for mt in range(4):
                        mg = half * 4 + mt
                        for kp in range(KP):
                            nc.tensor.matmul(pm[:, mg:mg + 1],
                                             mwt[:, 2 * kp:2 * kp + 2,
                                                 mt * P:(mt + 1) * P],
                                             condt[:, 2 * kp:2 * kp + 2, :],
                                             start=(kp == 0), stop=(kp == KP - 1),
                                             perf_mode=PM.DoubleRow)
                nc.vector.scalar_tensor_tensor(modv[:, w], pm[:, 0:KC], INV,
                                               modbt[:, w],
                                               AluOpType.mult, AluOpType.add)

            vecs = cp.tile([P, 6, KC], f32, tag="vecs")
            tgp = cp.tile([P, 2, KC], f32, tag="tgp")
            for w in range(2):
                mod_matvec(w)
            # scale1, shift1 (gate LN1 apply) from w0/w1 only
            nc.vector.tensor_scalar_add(tgp[:, 0], modv[:, 0], 1.0)
            nc.vector.tensor_tensor(vecs[:, 0], tgp[:, 0], lnt[:, 0], AluOpType.mult)
            nc.vector.tensor_tensor(vecs[:, 1], tgp[:, 0], lnt[:, 1], AluOpType.mult)
            nc.vector.tensor_tensor(vecs[:, 1], vecs[:, 1], modv[:, 1], AluOpType.add)
            # qkv weight block: ahead of the remaining modulation weights in
            # the SP queue so qkv can start on time
            qkA = ar.tile([P, KC, 2 * D], f8, tag="W4")
            nc.sync.dma_start(qkA[:],
                              qkvw[:, 0:2 * D].rearrange("(c p) m -> p c m", p=P))
            mod_matvec(2)
            nc.scalar.activation(vecs[:, 2], modv[:, 2], AF.Tanh)
            nc.vector.tensor_scalar_mul(vecs[:, 2], vecs[:, 2], GATE * INV)

            def late_mod():
                for w in range(3, 6):
                    mod_matvec(w)
                nc.vector.tensor_scalar_add(tgp[:, 1], modv[:, 3], 1.0)
                nc.vector.tensor_tensor(vecs[:, 3], tgp[:, 1], lnt[:, 2],
                                        AluOpType.mult)
                nc.vector.tensor_tensor(vecs[:, 4], tgp[:, 1], lnt[:, 3],
                                        AluOpType.mult)
                nc.vector.tensor_tensor(vecs[:, 4], vecs[:, 4], modv[:, 4],
                                        AluOpType.add)
                nc.scalar.activation(vecs[:, 5], modv[:, 5], AF.Tanh)
                nc.vector.tensor_scalar_mul(vecs[:, 5], vecs[:, 5], GATE * INV)

            def r32(tag="R32"):
                return rot.tile([P, NT], f32, tag=tag, bufs=4, name="r32")

            def r16(tag="R16", bufs=3):
                return rot.tile([P, NT], f16, tag=tag, bufs=bufs, name="r16")

            def layernorm(src16, ntok, scale_col, shift_col, out16):
                """src16(j) -> f16 [P, ntok] AP for stats+apply."""
                halves = ntok // 512
                pss = pbig()
                psq = pbig()
                for j in range(KC):
                    s16 = r16()
                    nc.gpsimd.tensor_tensor(s16[:, 0:ntok], src16(j),
                                            src16(j), AluOpType.mult)
                    for nh in range(halves):
                        sl = slice(nh * 512, (nh + 1) * 512)
                        nc.tensor.matmul(pss[:, sl], ones16[:], src16(j)[:, sl],
                                         start=(j == 0), stop=(j == KC - 1),
                                         skip_group_check=True)
                        nc.tensor.matmul(psq[:, sl], ones16[:], s16[:, sl],
                                         start=(j == 0), stop=(j == KC - 1),
                                         skip_group_check=True)
                murep = r32()
                nc.vector.tensor_scalar_mul(murep[:, 0:ntok], pss[:, 0:ntok],
                                            1.0 / D)
                msq = r32()
                nc.vector.tensor_scalar(msq[:, 0:ntok], psq[:, 0:ntok],
                                        1.0 / D, EPS,
                                        AluOpType.mult, AluOpType.add)
                mu2 = r32()
                nc.gpsimd.tensor_tensor(mu2[:, 0:ntok], murep[:, 0:ntok],
                                        murep[:, 0:ntok], AluOpType.mult)
                var = r32()
                nc.gpsimd.tensor_tensor(var[:, 0:ntok], msq[:, 0:ntok],
                                        mu2[:, 0:ntok], AluOpType.subtract)
                rvar = r32()
                nc.vector.reciprocal(rvar[:, 0:ntok], var[:, 0:ntok])
                arep = r32()
                nc.scalar.activation(arep[:, 0:ntok], rvar[:, 0:ntok], AF.Sqrt)
                mur16 = r16("MU16", 2)
                nc.gpsimd.tensor_copy(mur16[:, 0:ntok], murep[:, 0:ntok])
                ar16 = r16("MU16", 2)
                nc.gpsimd.tensor_copy(ar16[:, 0:ntok], arep[:, 0:ntok])
                for j in range(KC):
                    t1 = r16()
                    nc.gpsimd.tensor_tensor(t1[:, 0:ntok], src16(j),
                                            mur16[:, 0:ntok], AluOpType.subtract)
                    t2 = r16()
                    nc.gpsimd.tensor_tensor(t2[:, 0:ntok], t1[:, 0:ntok],
                                            ar16[:, 0:ntok], AluOpType.mult)
                    nc.vector.tensor_scalar(out16[:, j], t2[:, 0:ntok],
                                            vecs[:, scale_col, j:j + 1],
                                            vecs[:, shift_col, j:j + 1],
                                            AluOpType.mult, AluOpType.add)

            late_mod()

            # ---- S2/S3: LN1 + modulate (all 1024 tokens) ----
            y16 = ar.tile([P, KC, NT], f8, tag="Y2", bufs=2)
            layernorm(lambda j: xf[:, j], NT, 0, 1, y16)

            def dr_bias(ps, brow, mt, ncols=512):
                """Add bias (pair rows [b*scale; 0]) via K=1 fp8 DR matmul."""
                nc.tensor.matmul(ps[:], brow[:, :, mt * P:(mt + 1) * P],
                                 o2row[:, :, 0:ncols],
                                 start=False, stop=True,
                                 perf_mode=PM.DoubleRow,
                                 skip_group_check=True)

            # ---- S4+S5 interleaved: qkv / attention ----
            q16 = ar.tile([P, KC, LT], f8, tag="Q1", bufs=3)
            k16 = ar.tile([P, KC, NT], f8, tag="K2")
            v16 = ar.tile([P, KC, D], f8, tag="V2")
            vW = ar.tile([P, KC, D], f8, tag="Y2", bufs=2)
            nc.sync.dma_start(vW[:],
                              qkvw[:, 2 * D:3 * D].rearrange("(c p) m -> p c m", p=P))
            attn16 = ar.tile([P, KC, LT], f8, tag="AT")
            egs = {}

            def qk(g):
                pq = pp()
                for kp in range(KP):
                    nc.tensor.matmul(pq[:],
                                     qkA[:, 2 * kp:2 * kp + 2, g * P:(g + 1) * P],
                                     y16[:, 2 * kp:2 * kp + 2, 0:LT],
                                     start=(kp == 0), stop=False,
                                     perf_mode=PM.DoubleRow,
                                     skip_group_check=True)
                dr_bias(pq, qkbt, g)
                nc.vector.tensor_copy(q16[:, g], pq[:])
                for nh in range(2):
                    pk = pp()
                    for kp in range(KP):
                        nc.tensor.matmul(
                            pk[:],
                            qkA[:, 2 * kp:2 * kp + 2,
                                D + g * P:D + (g + 1) * P],
                            y16[:, 2 * kp:2 * kp + 2, nh * 512:(nh + 1) * 512],
                            start=(kp == 0), stop=False,
                            perf_mode=PM.DoubleRow,
                            skip_group_check=True)
                    dr_bias(pk, qkbt, 8 + g)
                    nc.vector.tensor_copy(k16[:, g, nh * 512:(nh + 1) * 512],
                                          pk[:])

            def sc(g):
                eg = ar.tile([P, KC, NT], f8, tag="EG", bufs=2)
                egs[g] = eg
                for c in range(KC):
                    psc = pbig()
                    nc.tensor.matmul(psc[:, 0:512],
                                     k16[0:DH, g, c * P:(c + 1) * P],
                                     q16[0:DH, g, :], start=True, stop=True,
                                     skip_group_check=True)
                    nc.tensor.matmul(psc[:, 512:1024],
                                     k16[DH:P, g, c * P:(c + 1) * P],
                                     q16[DH:P, g, :], start=True, stop=True,
                                     skip_group_check=True)
                    nc.scalar.activation(eg[:, c], psc[:], AF.Exp,
                                         scale=EXP_SCALE, bias=expb[:])

            def vblock(tt):
                for nh in range(2):
                    pv = pp()
                    for kp in range(KP):
                        nc.tensor.matmul(
                            pv[:],
                            y16[:, 2 * kp:2 * kp + 2, tt * P:(tt + 1) * P],
                            vW[:, 2 * kp:2 * kp + 2, nh * 512:(nh + 1) * 512],
                            start=(kp == 0), stop=False,
                            perf_mode=PM.DoubleRow,
                            skip_group_check=True)
                    # v bias: rhs holds [bv*WQ; 0] pair rows
                    nc.tensor.matmul(
                        pv[:],
                        ones_lhs[:, :, tt * P:(tt + 1) * P],
                        bvt[:, :, nh * 512:(nh + 1) * 512],
                        start=False, stop=True,
                        perf_mode=PM.DoubleRow, skip_group_check=True)
                    nc.vector.tensor_copy(
                        v16[:, tt, nh * 512:(nh + 1) * 512], pv[:])

            def rsav(g):
                eg = egs[g]
                pse = pbig()
                for cp2 in range(KP):
                    for nh in range(2):
                        nc.tensor.matmul(pse[:, nh * 512:(nh + 1) * 512],
                                         rso[:],
                                         eg[:, 2 * cp2:2 * cp2 + 2,
                                            nh * 512:(nh + 1) * 512],
                                         start=(cp2 == 0), stop=(cp2 == KP - 1),
                                         perf_mode=PM.DoubleRow,
                                         skip_group_check=True)
                recip = r32()
                nc.vector.reciprocal(recip[:], pse[:])
                # av: lhsT spans BOTH heads' v columns (M=128, no
                # tile_position); each psum holds one correct half (the
                # other half contracts the wrong head's weights and is
                # never read).
                pav1 = pp()
                pav2 = pp()
                for cp2 in range(KP):
                    vsl = v16[:, 2 * cp2:2 * cp2 + 2,
                              2 * g * DH:(2 * g + 2) * DH]
                    nc.tensor.matmul(pav1[:], vsl,
                                     eg[:, 2 * cp2:2 * cp2 + 2, 0:512],
                                     start=(cp2 == 0), stop=(cp2 == KP - 1),
                                     perf_mode=PM.DoubleRow,
                                     skip_group_check=True)
                    nc.tensor.matmul(pav2[:], vsl,
                                     eg[:, 2 * cp2:2 * cp2 + 2, 512:1024],
                                     start=(cp2 == 0), stop=(cp2 == KP - 1),
                                     perf_mode=PM.DoubleRow,
                                     skip_group_check=True)
                nc.vector.tensor_tensor(attn16[0:DH, g], pav1[0:DH, :],
                                        recip[0:DH, 0:512], AluOpType.mult)
                nc.vector.tensor_tensor(attn16[DH:P, g], pav2[DH:P, :],
                                        recip[DH:P, 512:1024], AluOpType.mult)

            qk(0); sc(0)
            qk(1); sc(1)
            for tt in range(KC):
                vblock(tt)
            rsav(0)
            for g in range(2, KC):
                qk(g); sc(g)
                rsav(g - 1)
            rsav(KC - 1)

            # ---- S6: proj + gated residual ----
            pw = ar.tile([P, KC, D], f8, tag="K2")
            nc.sync.dma_start(pw[:], projw.rearrange("(c p) m -> p c m", p=P))
            x2 = ar.tile([P, KC, LT], f32, tag="V2")
            for mt in range(KC):
                pj = pp()
                for kp in range(KP):
                    nc.tensor.matmul(pj[:],
                                     pw[:, 2 * kp:2 * kp + 2, mt * P:(mt + 1) * P],
                                     attn16[:, 2 * kp:2 * kp + 2, :],
                                     start=(kp == 0), stop=False,
                                     perf_mode=PM.DoubleRow,
                                     skip_group_check=True)
                dr_bias(pj, pbt, mt)
                nc.vector.scalar_tensor_tensor(x2[:, mt], pj[:],
                                               vecs[:, 2, mt:mt + 1],
                                               xf[:, mt, 0:LT],
                                               AluOpType.mult, AluOpType.add)

            # ---- S7: LN2 + modulate (local tokens) ----
            z16 = ar.tile([P, KC, LT], f8, tag="Q1", bufs=3)
            c2 = ar.tile([P, KC, LT], f16, tag="C2")
            for j in range(KC):
                nc.gpsimd.tensor_copy(c2[:, j], x2[:, j])
            layernorm(lambda j: c2[:, j], LT, 3, 4, z16)

            # ---- S8: fc1 + gelu ----
            h16 = ar.tile([P, 32, LT], f8, tag="A4")
            f1a = ar.tile([P, KC, 2 * D], f8, tag="W4")
            nc.sync.dma_start(f1a[:],
                              fc1w[:, 0:2 * D].rearrange("(c p) m -> p c m", p=P))
            f1b1 = ar.tile([P, KC, D], f8, tag="K2")
            nc.sync.dma_start(f1b1[:],
                              fc1w[:, 2 * D:3 * D].rearrange("(c p) m -> p c m", p=P))

            def fc1_block(wt, mg0, nmt):
                for mt in range(nmt):
                    mg = mg0 + mt
                    ph = pp()
                    for kp in range(KP):
                        nc.tensor.matmul(ph[:],
                                         wt[:, 2 * kp:2 * kp + 2,
                                            mt * P:(mt + 1) * P],
                                         z16[:, 2 * kp:2 * kp + 2, :],
                                         start=(kp == 0), stop=False,
                                         perf_mode=PM.DoubleRow,
                                         skip_group_check=True)
                    dr_bias(ph, f1bt, mg)
                    nc.scalar.activation(h16[:, mg], ph[:], AF.Gelu, scale=INV)

            fc1_block(f1a, 0, 16)
            f1b2t = ar.tile([P, KC, D], f8, tag="W4")
            nc.sync.dma_start(f1b2t[:],
                              fc1w[:, 3 * D:4 * D].rearrange("(c p) m -> p c m", p=P))
            fc1_block(f1b1, 16, 8)
            fc1_block(f1b2t, 24, 8)

            # ---- S9: fc2 + gated residual + store ----
            for mt in range(KC):
                f2col = ar.tile([P, 32, P], f8, tag="Q1", bufs=3)
                nc.sync.dma_start(
                    f2col[:],
                    fc2w[mt * P:(mt + 1) * P, :]
                    .rearrange("p (c m) -> p c m", m=P))
                pz = pp()
                for kp in range(16):
                    nc.tensor.matmul(pz[:], f2col[:, 2 * kp:2 * kp + 2, :],
                                     h16[:, 2 * kp:2 * kp + 2, :],
                                     start=(kp == 0), stop=False,
                                     perf_mode=PM.DoubleRow,
                                     skip_group_check=True)
                dr_bias(pz, f2bt, mt)
                ot = rot.tile([P, LT], f32, tag="OT", bufs=2)
                nc.vector.scalar_tensor_tensor(ot[:], pz[:],
                                               vecs[:, 5, mt:mt + 1],
                                               x2[:, mt, :],
                                               AluOpType.mult, AluOpType.add)
                nc.sync.dma_start(outt[mt * P:(mt + 1) * P, :], ot[:])

    if legalize:
        _legalize_waits(nc)
    return nc


_NC_CACHE = {}


def _get_nc():
    if "nc" not in _NC_CACHE:
        _NC_CACHE["nc"] = _build()
    return _NC_CACHE["nc"]


def _feat(v, cols):
    """[D*]-vector -> feature-major [128, cols] (col j = chunk j)."""
    return np.ascontiguousarray(v.reshape(cols, P).T)


def _b2(v, scale, f8):
    """bias vector -> [1, 2, len] fp8 pair rows [b*scale; 0]."""
    n = v.shape[0]
    out = np.zeros((1, 2, n), np.float32)
    out[0, 0] = np.asarray(v, np.float32) * scale
    return out.astype(f8)


def make_in_maps(x, cond, g1_w, g1_b, b1_w, b1_b, a1_w, a1_b,
                 g2_w, g2_b, b2_w, b2_b, a2_w, a2_b,
                 ln1_g, ln1_b, ln2_g, ln2_b,
                 qkv_w, qkv_b, proj_w, proj_b,
                 fc1_w, fc1_b, fc2_w, fc2_b):
    f32 = np.float32
    f16 = np.float16
    f8 = dt.np(dt.float8e4)
    x = np.asarray(x, f32)
    cond = np.asarray(cond, f32)

    def w8(w, s):
        return (np.asarray(w, f32) * s).astype(f8)

    shared = {
        "qkvw": w8(qkv_w, WQ),
        "qkb2": _b2(np.asarray(qkv_b, f32)[0:2 * D], WQ, f8),
        "bv2": _b2(np.asarray(qkv_b, f32)[2 * D:3 * D], WQ, f8),
        "projw": w8(proj_w, WS),
        "pb2": _b2(np.asarray(proj_b, f32), WS, f8),
        "fc1w": w8(fc1_w, WS),
        "f1b2": _b2(np.asarray(fc1_b, f32), WS, f8),
        # [mt*128+p, kc*128+m] = fc2_w[kc*128+p, mt*128+m]: contiguous
        # per-mt loads of the feature-major lhsT tiles
        "fc2w": np.ascontiguousarray(
            w8(fc2_w, WS).reshape(32, P, KC, P)
            .transpose(2, 1, 0, 3).reshape(D, DFF)),
        "f2b2": _b2(np.asarray(fc2_b, f32), WS, f8),
        # row (w*2+half)*128+p, col kc*512+m = W_w[kc*128+p, half*512+m]:
        # contiguous loads of each feature-major half-block
        "modw": np.ascontiguousarray(
            np.hstack([(np.asarray(w, f32) * WS) for w in
                       (g1_w, b1_w, a1_w, g2_w, b2_w, a2_w)])
            .reshape(KC, P, 6, 2, 512).transpose(2, 3, 1, 0, 4)
            .reshape(12 * P, 4 * D)).astype(f8),
        "modbf": np.hstack([_feat(np.asarray(v, f32), KC) for v in
                            (g1_b, b1_b, a1_b, g2_b, b2_b, a2_b)]),
        "lnf": np.hstack([_feat(np.asarray(v, f32), KC) for v in
                          (ln1_g, ln1_b, ln2_g, ln2_b)]),
    }
    in_maps = []
    for c in range(8):
        b, h = c // 2, c % 2
        xb = x[b].T  # [D, NT]
        perm = np.concatenate([np.arange(h * LT, (h + 1) * LT),
                               np.arange((1 - h) * LT, (2 - h) * LT)])
        m = dict(shared)
        m["xt"] = np.ascontiguousarray(xb[:, perm]).astype(f16)
        m["cond8"] = _feat(cond[b], KC).astype(f8)[:, :, None]
        in_maps.append(m)
    return in_maps


def kernel(**inputs):
    nc = _get_nc()
    in_maps = make_in_maps(**inputs)
    res = run_bass_kernel_spmd(nc, in_maps, list(range(8)))
    out = np.empty((B, NT, D), np.float32)
    for c in range(8):
        b, h = c // 2, c % 2
        out[b, h * LT:(h + 1) * LT, :] = res.results[c]["outt"].T
    return out


# revision 11
# speedup vs baseline: 1.1735x; 1.1735x over previous
"""DiT block kernel for 8 trn2 NeuronCores — fp8 DoubleRow version.

Sharding: core c -> (batch b=c//2, query-token half h=c%2). Each core
computes the full block for its 512 query tokens (K/V compute for all
1024 tokens of its batch is replicated within the pair) -> zero
collectives. Activations are feature-major ([D on partitions, tokens on
free]); weights are used in natural [in, out] layout as matmul lhsT.

All large matmuls run in fp8 e4m3 with MatmulPerfMode.DoubleRow (two
128-deep K slices per instruction, 2x PE throughput, half the weight
DMA). Weights are pre-scaled on host (x32 for qkv so the stored q/k/v
fit e4m3 range, x64 elsewhere) so values sit in e4m3's normal range;
corrections fold into the exp scale (1/1024), the softmax rowsum ones
constant (32.0) and the residual gates (1/64) — q/k/v epilogues are
pure psum->fp8 copies. Biases enter psum via K=1 fp8 DoubleRow matmuls
([bias*scale; 0] pair rows against an all-ones moving row).

qkv and attention are interleaved (qk(g), scores(g) emitted per head
pair, v blocks before the first av) so the PE stays busy while ACT
streams the 64 softmax exps. LN stats/softmax normalization/residuals
stay fp32/f16; psum reads go to DVE, f16 SBUF work to Pool(gpsimd),
exp/gelu/tanh/sqrt to ACT.
"""
import numpy as np

import concourse.bass as bass
import concourse.tile as tile
import concourse.mybir as mybir
from concourse.bass_utils import run_bass_kernel_spmd
from concourse.vector_clock import ScopedClock
from concourse.alu_op_type import AluOpType

dt = mybir.dt
AF = mybir.ActivationFunctionType
PM = mybir.MatmulPerfMode

P = 128
B, NT, D, H = 4, 1024, 1024, 16
DH = D // H            # 64
DFF = 4 * D            # 4096
KC = D // P            # 8
KP = KC // 2           # 4 DoubleRow K-pairs
LT = NT // 2           # 512 local query tokens
GATE = 0.1
EPS = 1e-5
EXP_BIAS = -3.0        # constant shift inside exp; cancels in softmax
WQ = 32.0              # host pre-scale for qkv weights (q/k/v stored x32)
WS = 64.0              # host pre-scale for the other weights
INV = 1.0 / WS
EXP_SCALE = DH ** -0.5 / (WQ * WQ)   # scores psum holds (32q)·(32k)
RS_ONES = WQ           # rowsum ones constant: recip absorbs v's x32


class SplitDrainTileContext(tile.TileContext):
    """Tail drain in this walrus build holds few sync waits; spill the
    rest onto chained SP nops (runs before the sem-clear barrier, so
    semantics are preserved)."""

    MAX_TAIL_WAITS = 1

    def _drain_and_barrier(self, tick_clock, wait_clock):
        drain_inst = self.nc.sync.drain()
        wait_clock.add_sem_waits(
            drain_inst.ins, ScopedClock({None: tick_clock.global_clock})
        )
        si = drain_inst.ins.sync_info
        waits = list(si.on_wait) if si else []
        if len(waits) > self.MAX_TAIL_WAITS:
            drain_inst.ins.sync_info = mybir.SyncInfo(
                on_wait=waits[: self.MAX_TAIL_WAITS],
                on_update=list(si.on_update) if si else [],
            )
            rest = waits[self.MAX_TAIL_WAITS:]
            for i in range(0, len(rest), self.MAX_TAIL_WAITS):
                nop = self.nc.sync.nop()
                nop.ins.sync_info = mybir.SyncInfo(
                    on_wait=rest[i : i + self.MAX_TAIL_WAITS], on_update=[]
                )
        self.nc.all_engine_barrier()
        assert self.sems is not None
        popped = self.nc._tile_sem_poison_stack.pop()
        assert popped is self._sem_poison
        self.nc.clear_and_free_semaphores(list(self.sems.allocated().values()))
        self.nc.all_engine_barrier()


def _legalize_waits(nc, max_waits=1):
    """This walrus build accepts at most one sync wait per instruction.
    Move surplus waits onto same-engine NoOps inserted just before the
    offending instruction (engine FIFO order preserves semantics)."""
    fix = 0
    for bb in nc.main_func.blocks:
        insts = list(bb.instructions)
        out = []
        for inst in insts:
            si = inst.sync_info
            waits = list(si.on_wait) if si else []
            if len(waits) > max_waits:
                keep = waits[-max_waits:]
                for w in waits[:-max_waits]:
                    nop = mybir.InstNoOp(name=f"I-wfix{fix}")
                    fix += 1
                    nop.engine = inst.engine
                    nop.sync_info = mybir.SyncInfo(on_wait=[w], on_update=[])
                    out.append(nop)
                inst.sync_info = mybir.SyncInfo(
                    on_wait=keep, on_update=list(si.on_update) if si else [])
            out.append(inst)
        if len(out) != len(insts):
            bb.instructions = out
    return fix


def _build(legalize=True, use_pool=None):
    import os
    if use_pool is None:
        use_pool = not os.environ.get("V2_NO_POOL")
    nc = bass.Bass(target_bir_lowering=False, debug=False,
                   dynamic_dma_scratch_size=2048)
    f32, f16, f8 = dt.float32, dt.float16, dt.float8e4

    xt = nc.dram_tensor("xt", [D, NT], f16, kind="ExternalInput")
    cond8 = nc.dram_tensor("cond8", [P, KC, 1], f8, kind="ExternalInput")
    qkvw = nc.dram_tensor("qkvw", [D, 3 * D], f8, kind="ExternalInput")
    qkb2 = nc.dram_tensor("qkb2", [1, 2, 2 * D], f8, kind="ExternalInput")
    bv2 = nc.dram_tensor("bv2", [1, 2, D], f8, kind="ExternalInput")
    projw = nc.dram_tensor("projw", [D, D], f8, kind="ExternalInput")
    pb2 = nc.dram_tensor("pb2", [1, 2, D], f8, kind="ExternalInput")
    fc1w = nc.dram_tensor("fc1w", [D, DFF], f8, kind="ExternalInput")
    f1b2 = nc.dram_tensor("f1b2", [1, 2, DFF], f8, kind="ExternalInput")
    fc2w = nc.dram_tensor("fc2w", [D, DFF], f8, kind="ExternalInput")  # host-rearranged
    f2b2 = nc.dram_tensor("f2b2", [1, 2, D], f8, kind="ExternalInput")
    modw = nc.dram_tensor("modw", [12 * P, 4 * D], f8, kind="ExternalInput")  # host-rearranged
    modbf = nc.dram_tensor("modbf", [P, 6 * KC], f32, kind="ExternalInput")
    lnf = nc.dram_tensor("lnf", [P, 4 * KC], f32, kind="ExternalInput")
    outt = nc.dram_tensor("outt", [D, LT], f32, kind="ExternalOutput")

    gp = nc.gpsimd if use_pool else nc.vector
    with SplitDrainTileContext(nc) as tc:
        with tc.tile_pool(name="cp", bufs=1) as cp, \
             tc.tile_pool(name="ar", bufs=1) as ar, \
             tc.tile_pool(name="rot", bufs=4) as rot, \
             tc.tile_pool(name="psA", bufs=4, space="PSUM") as psA, \
             tc.tile_pool(name="psB", bufs=2, space="PSUM") as psB:

            def pp():    # [P, 512] f32 psum, 4 rotating banks
                return psA.tile([P, 512], f32, tag="pp", name="pp")

            def pbig():  # [P, 1024] f32 psum, 2 rotating 2-bank tiles
                return psB.tile([P, 1024], f32, tag="big", name="big")

            ones16 = cp.tile([P, P], f16, tag="ones16")
            nc.vector.memset(ones16[:], 1.0)
            rso = cp.tile([P, 2, P], f8, tag="rso")
            nc.vector.memset(rso[:], RS_ONES)
            o2row = cp.tile([1, 2, LT], f8, tag="o2row")
            nc.vector.memset(o2row[:], 1.0)
            ones_lhs = cp.tile([1, 2, D], f8, tag="ones_lhs")
            nc.vector.memset(ones_lhs[:], 1.0)
            expb = cp.tile([P, 1], f32, tag="expb")
            nc.vector.memset(expb[:], EXP_BIAS)

            # ---- resident small inputs ----
            condt = cp.tile([P, KC, 1], f8, tag="condt")
            nc.sync.dma_start(condt[:], cond8[:])
            lnt = cp.tile([P, 4, KC], f32, tag="lnt")
            nc.sync.dma_start(lnt[:], lnf.rearrange("p (w c) -> p w c", c=KC))
            modbt = cp.tile([P, 6, KC], f32, tag="modbt")
            nc.sync.dma_start(modbt[:], modbf.rearrange("p (w c) -> p w c", c=KC))
            qkbt = cp.tile([1, 2, 2 * D], f8, tag="qkbt")
            nc.sync.dma_start(qkbt[:], qkb2[:])
            bvt = cp.tile([1, 2, D], f8, tag="bvt")
            nc.sync.dma_start(bvt[:], bv2[:])
            pbt = cp.tile([1, 2, D], f8, tag="pbt")
            nc.sync.dma_start(pbt[:], pb2[:])
            f1bt = cp.tile([1, 2, DFF], f8, tag="f1bt")
            nc.sync.dma_start(f1bt[:], f1b2[:])
            f2bt = cp.tile([1, 2, D], f8, tag="f2bt")
            nc.sync.dma_start(f2bt[:], f2b2[:])

            # x, feature-major f16, 2 MiB (tag A4 later reused by h16)
            xf = ar.tile([P, KC, NT], f16, tag="A4")
            nc.sync.dma_start(xf[:], xt.rearrange("(c p) t -> p c t", p=P))

            # ---- S1: modulation matvecs (feature-major out) ----
            modv = cp.tile([P, 6, KC], f32, tag="modv")

            def mod_matvec(w):
                pm = pp()  # [P, 8] accum lives in a [P,512] slot
                for half in range(2):
                    mwt = ar.tile([P, KC, 512], f8, tag="Q1", bufs=3)
                    nc.sync.dma_start(
                        mwt[:],
                        modw[(w * 2 + half) * P:(w * 2 + half + 1) * P, :]
                        .rearrange("p (c m) -> p c m", m=512))
                    for mt in range(4):
                        mg = half * 4 + mt
                        for kp in range(KP):
                            nc.tensor.matmul(pm[:, mg:mg + 1],
                                             mwt[:, 2 * kp:2 * kp + 2,
                                                 mt * P:(mt + 1) * P],
                                             condt[:, 2 * kp:2 * kp + 2, :],
                                             start=(kp == 0), stop=(kp == KP - 1),
                                             perf_mode=PM.DoubleRow)
                nc.vector.scalar_tensor_tensor(modv[:, w], pm[:, 0:KC], INV,
                                               modbt[:, w],
                                               AluOpType.mult, AluOpType.add)

            vecs = cp.tile([P, 6, KC], f32, tag="vecs")
            tgp = cp.tile([P, 2, KC], f32, tag="tgp")
            for w in range(2):
                mod_matvec(w)
            # scale1, shift1 (gate LN1 apply) from w0/w1 only
            nc.vector.tensor_scalar_add(tgp[:, 0], modv[:, 0], 1.0)
            nc.vector.tensor_tensor(vecs[:, 0], tgp[:, 0], lnt[:, 0], AluOpType.mult)
            nc.vector.tensor_tensor(vecs[:, 1], tgp[:, 0], lnt[:, 1], AluOpType.mult)
            nc.vector.tensor_tensor(vecs[:, 1], vecs[:, 1], modv[:, 1], AluOpType.add)
            # qkv weight block: ahead of the remaining modulation weights in
            # the SP queue so qkv can start on time
            qkA = ar.tile([P, KC, 2 * D], f8, tag="W4")
            nc.sync.dma_start(qkA[:],
                              qkvw[:, 0:2 * D].rearrange("(c p) m -> p c m", p=P))
            mod_matvec(2)
            nc.scalar.activation(vecs[:, 2], modv[:, 2], AF.Tanh)
            nc.vector.tensor_scalar_mul(vecs[:, 2], vecs[:, 2], GATE * INV)

            def late_mod():
                for w in range(3, 6):
                    mod_matvec(w)
                nc.vector.tensor_scalar_add(tgp[:, 1], modv[:, 3], 1.0)
                nc.vector.tensor_tensor(vecs[:, 3], tgp[:, 1], lnt[:, 2],
                                        AluOpType.mult)
                nc.vector.tensor_tensor(vecs[:, 4], tgp[:, 1], lnt[:, 3],
                                        AluOpType.mult)
                nc.vector.tensor_tensor(vecs[:, 4], vecs[:, 4], modv[:, 4],
                                        AluOpType.add)
                nc.scalar.activation(vecs[:, 5], modv[:, 5], AF.Tanh)
                nc.vector.tensor_scalar_mul(vecs[:, 5], vecs[:, 5], GATE * INV)

            def r32(tag="R32"):
                return rot.tile([P, NT], f32, tag=tag, bufs=4, name="r32")

            def r16(tag="R16", bufs=3):
                return rot.tile([P, NT], f16, tag=tag, bufs=bufs, name="r16")

            def layernorm(src16, ntok, scale_col, shift_col, out16):
                """src16(j) -> f16 [P, ntok] AP for stats+apply."""
                halves = ntok // 512
                pss = pbig()
                psq = pbig()
                for j in range(KC):
                    s16 = r16()
                    gp.tensor_tensor(s16[:, 0:ntok], src16(j),
                                            src16(j), AluOpType.mult)
                    for nh in range(halves):
                        sl = slice(nh * 512, (nh + 1) * 512)
                        nc.tensor.matmul(pss[:, sl], ones16[:], src16(j)[:, sl],
                                         start=(j == 0), stop=(j == KC - 1),
                                         skip_group_check=True)
                        nc.tensor.matmul(psq[:, sl], ones16[:], s16[:, sl],
                                         start=(j == 0), stop=(j == KC - 1),
                                         skip_group_check=True)
                murep = r32()
                nc.vector.tensor_scalar_mul(murep[:, 0:ntok], pss[:, 0:ntok],
                                            1.0 / D)
                msq = r32()
                nc.vector.tensor_scalar(msq[:, 0:ntok], psq[:, 0:ntok],
                                        1.0 / D, EPS,
                                        AluOpType.mult, AluOpType.add)
                mu2 = r32()
                gp.tensor_tensor(mu2[:, 0:ntok], murep[:, 0:ntok],
                                        murep[:, 0:ntok], AluOpType.mult)
                var = r32()
                gp.tensor_tensor(var[:, 0:ntok], msq[:, 0:ntok],
                                        mu2[:, 0:ntok], AluOpType.subtract)
                rvar = r32()
                nc.vector.reciprocal(rvar[:, 0:ntok], var[:, 0:ntok])
                arep = r32()
                nc.scalar.activation(arep[:, 0:ntok], rvar[:, 0:ntok], AF.Sqrt)
                mur16 = r16("MU16", 2)
                gp.tensor_copy(mur16[:, 0:ntok], murep[:, 0:ntok])
                ar16 = r16("MU16", 2)
                gp.tensor_copy(ar16[:, 0:ntok], arep[:, 0:ntok])
                for j in range(KC):
                    t1 = r16()
                    gp.tensor_tensor(t1[:, 0:ntok], src16(j),
                                            mur16[:, 0:ntok], AluOpType.subtract)
                    t2 = r16()
                    gp.tensor_tensor(t2[:, 0:ntok], t1[:, 0:ntok],
                                            ar16[:, 0:ntok], AluOpType.mult)
                    nc.vector.tensor_scalar(out16[:, j], t2[:, 0:ntok],
                                            vecs[:, scale_col, j:j + 1],
                                            vecs[:, shift_col, j:j + 1],
                                            AluOpType.mult, AluOpType.add)

            late_mod()

            # ---- S2/S3: LN1 + modulate (all 1024 tokens) ----
            y16 = ar.tile([P, KC, NT], f8, tag="Y2", bufs=2)
            layernorm(lambda j: xf[:, j], NT, 0, 1, y16)

            def dr_bias(ps, brow, mt, ncols=512):
                """Add bias (pair rows [b*scale; 0]) via K=1 fp8 DR matmul."""
                nc.tensor.matmul(ps[:], brow[:, :, mt * P:(mt + 1) * P],
                                 o2row[:, :, 0:ncols],
                                 start=False, stop=True,
                                 perf_mode=PM.DoubleRow,
                                 skip_group_check=True)

            # ---- S4+S5 interleaved: qkv / attention ----
            q16 = ar.tile([P, KC, LT], f8, tag="Q1", bufs=3)
            k16 = ar.tile([P, KC, NT], f8, tag="K2")
            v16 = ar.tile([P, KC, D], f8, tag="V2")
            vW = ar.tile([P, KC, D], f8, tag="Y2", bufs=2)
            nc.sync.dma_start(vW[:],
                              qkvw[:, 2 * D:3 * D].rearrange("(c p) m -> p c m", p=P))
            attn16 = ar.tile([P, KC, LT], f8, tag="AT")
            egs = {}

            def qk(g):
                pq = pp()
                for kp in range(KP):
                    nc.tensor.matmul(pq[:],
                                     qkA[:, 2 * kp:2 * kp + 2, g * P:(g + 1) * P],
                                     y16[:, 2 * kp:2 * kp + 2, 0:LT],
                                     start=(kp == 0), stop=False,
                                     perf_mode=PM.DoubleRow,
                                     skip_group_check=True)
                dr_bias(pq, qkbt, g)
                nc.vector.tensor_copy(q16[:, g], pq[:])
                for nh in range(2):
                    pk = pp()
                    for kp in range(KP):
                        nc.tensor.matmul(
                            pk[:],
                            qkA[:, 2 * kp:2 * kp + 2,
                                D + g * P:D + (g + 1) * P],
                            y16[:, 2 * kp:2 * kp + 2, nh * 512:(nh + 1) * 512],
                            start=(kp == 0), stop=False,
                            perf_mode=PM.DoubleRow,
                            skip_group_check=True)
                    dr_bias(pk, qkbt, 8 + g)
                    nc.vector.tensor_copy(k16[:, g, nh * 512:(nh + 1) * 512],
                                          pk[:])

            def sc(g):
                eg = ar.tile([P, KC, NT], f8, tag="EG", bufs=2)
                egs[g] = eg
                for c in range(KC):
                    psc = pbig()
                    nc.tensor.matmul(psc[:, 0:512],
                                     k16[0:DH, g, c * P:(c + 1) * P],
                                     q16[0:DH, g, :], start=True, stop=True,
                                     skip_group_check=True)
                    nc.tensor.matmul(psc[:, 512:1024],
                                     k16[DH:P, g, c * P:(c + 1) * P],
                                     q16[DH:P, g, :], start=True, stop=True,
                                     skip_group_check=True)
                    nc.scalar.activation(eg[:, c], psc[:], AF.Exp,
                                         scale=EXP_SCALE, bias=expb[:])

            def vblock(tt):
                for nh in range(2):
                    pv = pp()
                    for kp in range(KP):
                        nc.tensor.matmul(
                            pv[:],
                            y16[:, 2 * kp:2 * kp + 2, tt * P:(tt + 1) * P],
                            vW[:, 2 * kp:2 * kp + 2, nh * 512:(nh + 1) * 512],
                            start=(kp == 0), stop=False,
                            perf_mode=PM.DoubleRow,
                            skip_group_check=True)
                    # v bias: rhs holds [bv*WQ; 0] pair rows
                    nc.tensor.matmul(
                        pv[:],
                        ones_lhs[:, :, tt * P:(tt + 1) * P],
                        bvt[:, :, nh * 512:(nh + 1) * 512],
                        start=False, stop=True,
                        perf_mode=PM.DoubleRow, skip_group_check=True)
                    nc.vector.tensor_copy(
                        v16[:, tt, nh * 512:(nh + 1) * 512], pv[:])

            def rsav(g):
                eg = egs[g]
                pse = pbig()
                for cp2 in range(KP):
                    for nh in range(2):
                        nc.tensor.matmul(pse[:, nh * 512:(nh + 1) * 512],
                                         rso[:],
                                         eg[:, 2 * cp2:2 * cp2 + 2,
                                            nh * 512:(nh + 1) * 512],
                                         start=(cp2 == 0), stop=(cp2 == KP - 1),
                                         perf_mode=PM.DoubleRow,
                                         skip_group_check=True)
                recip = r32()
                nc.vector.reciprocal(recip[:], pse[:])
                # av: lhsT spans BOTH heads' v columns (M=128, no
                # tile_position); each psum holds one correct half (the
                # other half contracts the wrong head's weights and is
                # never read).
                pav1 = pp()
                pav2 = pp()
                for cp2 in range(KP):
                    vsl = v16[:, 2 * cp2:2 * cp2 + 2,
                              2 * g * DH:(2 * g + 2) * DH]
                    nc.tensor.matmul(pav1[:], vsl,
                                     eg[:, 2 * cp2:2 * cp2 + 2, 0:512],
                                     start=(cp2 == 0), stop=(cp2 == KP - 1),
                                     perf_mode=PM.DoubleRow,
                                     skip_group_check=True)
                    nc.tensor.matmul(pav2[:], vsl,
                                     eg[:, 2 * cp2:2 * cp2 + 2, 512:1024],
                                     start=(cp2 == 0), stop=(cp2 == KP - 1),
                                     perf_mode=PM.DoubleRow,
                                     skip_group_check=True)
                nc.vector.tensor_tensor(attn16[0:DH, g], pav1[0:DH, :],
                                        recip[0:DH, 0:512], AluOpType.mult)
                nc.vector.tensor_tensor(attn16[DH:P, g], pav2[DH:P, :],
                                        recip[DH:P, 512:1024], AluOpType.mult)

            qk(0); sc(0)
            qk(1); sc(1)
            for tt in range(KC):
                vblock(tt)
            rsav(0)
            for g in range(2, KC):
                qk(g); sc(g)
                rsav(g - 1)
            rsav(KC - 1)

            # ---- S6: proj + gated residual ----
            pw = ar.tile([P, KC, D], f8, tag="K2")
            nc.sync.dma_start(pw[:], projw.rearrange("(c p) m -> p c m", p=P))
            x2 = ar.tile([P, KC, LT], f32, tag="V2")
            for mt in range(KC):
                pj = pp()
                for kp in range(KP):
                    nc.tensor.matmul(pj[:],
                                     pw[:, 2 * kp:2 * kp + 2, mt * P:(mt + 1) * P],
                                     attn16[:, 2 * kp:2 * kp + 2, :],
                                     start=(kp == 0), stop=False,
                                     perf_mode=PM.DoubleRow,
                                     skip_group_check=True)
                dr_bias(pj, pbt, mt)
                nc.vector.scalar_tensor_tensor(x2[:, mt], pj[:],
                                               vecs[:, 2, mt:mt + 1],
                                               xf[:, mt, 0:LT],
                                               AluOpType.mult, AluOpType.add)

            # ---- S7: LN2 + modulate (local tokens) ----
            z16 = ar.tile([P, KC, LT], f8, tag="Q1", bufs=3)
            c2 = ar.tile([P, KC, LT], f16, tag="C2")
            for j in range(KC):
                gp.tensor_copy(c2[:, j], x2[:, j])
            layernorm(lambda j: c2[:, j], LT, 3, 4, z16)

            # ---- S8: fc1 + gelu ----
            h16 = ar.tile([P, 32, LT], f8, tag="A4")
            f1a = ar.tile([P, KC, 2 * D], f8, tag="W4")
            nc.sync.dma_start(f1a[:],
                              fc1w[:, 0:2 * D].rearrange("(c p) m -> p c m", p=P))
            f1b1 = ar.tile([P, KC, D], f8, tag="K2")
            nc.sync.dma_start(f1b1[:],
                              fc1w[:, 2 * D:3 * D].rearrange("(c p) m -> p c m", p=P))

            def fc1_block(wt, mg0, nmt):
                for mt in range(nmt):
                    mg = mg0 + mt
                    ph = pp()
                    for kp in range(KP):
                        nc.tensor.matmul(ph[:],
                                         wt[:, 2 * kp:2 * kp + 2,
                                            mt * P:(mt + 1) * P],
                                         z16[:, 2 * kp:2 * kp + 2, :],
                                         start=(kp == 0), stop=False,
                                         perf_mode=PM.DoubleRow,
                                         skip_group_check=True)
                    dr_bias(ph, f1bt, mg)
                    nc.scalar.activation(h16[:, mg], ph[:], AF.Gelu, scale=INV)

            fc1_block(f1a, 0, 16)
            f1b2t = ar.tile([P, KC, D], f8, tag="W4")
            nc.sync.dma_start(f1b2t[:],
                              fc1w[:, 3 * D:4 * D].rearrange("(c p) m -> p c m", p=P))
            fc1_block(f1b1, 16, 8)
            fc1_block(f1b2t, 24, 8)

            # ---- S9: fc2 + gated residual + store ----
            for mt in range(KC):
                f2col = ar.tile([P, 32, P], f8, tag="Q1", bufs=3)
                nc.sync.dma_start(
                    f2col[:],
                    fc2w[mt * P:(mt + 1) * P, :]
                    .rearrange("p (c m) -> p c m", m=P))
                pz = pp()
                for kp in range(16):
                    nc.tensor.matmul(pz[:], f2col[:, 2 * kp:2 * kp + 2, :],
                                     h16[:, 2 * kp:2 * kp + 2, :],
                                     start=(kp == 0), stop=False,
                                     perf_mode=PM.DoubleRow,
                                     skip_group_check=True)
                dr_bias(pz, f2bt, mt)
                ot = rot.tile([P, LT], f32, tag="OT", bufs=2)
                nc.vector.scalar_tensor_tensor(ot[:], pz[:],
                                               vecs[:, 5, mt:mt + 1],
                                               x2[:, mt, :],
                                               AluOpType.mult, AluOpType.add)
                nc.sync.dma_start(outt[mt * P:(mt + 1) * P, :], ot[:])

    if legalize:
        _legalize_waits(nc)
    return nc


_NC_CACHE = {}


def _get_nc():
    if "nc" not in _NC_CACHE:
        _NC_CACHE["nc"] = _build()
    return _NC_CACHE["nc"]


def _feat(v, cols):
    """[D*]-vector -> feature-major [128, cols] (col j = chunk j)."""
    return np.ascontiguousarray(v.reshape(cols, P).T)


def _b2(v, scale, f8):
    """bias vector -> [1, 2, len] fp8 pair rows [b*scale; 0]."""
    n = v.shape[0]
    out = np.zeros((1, 2, n), np.float32)
    out[0, 0] = np.asarray(v, np.float32) * scale
    return out.astype(f8)


def make_in_maps(x, cond, g1_w, g1_b, b1_w, b1_b, a1_w, a1_b,
                 g2_w, g2_b, b2_w, b2_b, a2_w, a2_b,
                 ln1_g, ln1_b, ln2_g, ln2_b,
                 qkv_w, qkv_b, proj_w, proj_b,
                 fc1_w, fc1_b, fc2_w, fc2_b):
    f32 = np.float32
    f16 = np.float16
    f8 = dt.np(dt.float8e4)
    x = np.asarray(x, f32)
    cond = np.asarray(cond, f32)

    def w8(w, s):
        return (np.asarray(w, f32) * s).astype(f8)

    shared = {
        "qkvw": w8(qkv_w, WQ),
        "qkb2": _b2(np.asarray(qkv_b, f32)[0:2 * D], WQ, f8),
        "bv2": _b2(np.asarray(qkv_b, f32)[2 * D:3 * D], WQ, f8),
        "projw": w8(proj_w, WS),
        "pb2": _b2(np.asarray(proj_b, f32), WS, f8),
        "fc1w": w8(fc1_w, WS),
        "f1b2": _b2(np.asarray(fc1_b, f32), WS, f8),
        # [mt*128+p, kc*128+m] = fc2_w[kc*128+p, mt*128+m]: contiguous
        # per-mt loads of the feature-major lhsT tiles
        "fc2w": np.ascontiguousarray(
            w8(fc2_w, WS).reshape(32, P, KC, P)
            .transpose(2, 1, 0, 3).reshape(D, DFF)),
        "f2b2": _b2(np.asarray(fc2_b, f32), WS, f8),
        # row (w*2+half)*128+p, col kc*512+m = W_w[kc*128+p, half*512+m]:
        # contiguous loads of each feature-major half-block
        "modw": np.ascontiguousarray(
            np.hstack([(np.asarray(w, f32) * WS) for w in
                       (g1_w, b1_w, a1_w, g2_w, b2_w, a2_w)])
            .reshape(KC, P, 6, 2, 512).transpose(2, 3, 1, 0, 4)
            .reshape(12 * P, 4 * D)).astype(f8),
        "modbf": np.hstack([_feat(np.asarray(v, f32), KC) for v in
                            (g1_b, b1_b, a1_b, g2_b, b2_b, a2_b)]),
        "lnf": np.hstack([_feat(np.asarray(v, f32), KC) for v in
                          (ln1_g, ln1_b, ln2_g, ln2_b)]),
    }
    in_maps = []
    for c in range(8):
        b, h = c // 2, c % 2
        xb = x[b].T  # [D, NT]
        perm = np.concatenate([np.arange(h * LT, (h + 1) * LT),
                               np.arange((1 - h) * LT, (2 - h) * LT)])
        m = dict(shared)
        m["xt"] = np.ascontiguousarray(xb[:, perm]).astype(f16)
        m["cond8"] = _feat(cond[b], KC).astype(f8)[:, :, None]
        in_maps.append(m)
    return in_maps


def kernel(**inputs):
    nc = _get_nc()
    in_maps = make_in_maps(**inputs)
    res = run_bass_kernel_spmd(nc, in_maps, list(range(8)))
    out = np.empty((B, NT, D), np.float32)
    for c in range(8):
        b, h = c // 2, c % 2
        out[b, h * LT:(h + 1) * LT, :] = res.results[c]["outt"].T
    return out


# revision 13
# speedup vs baseline: 1.4486x; 1.2344x over previous
"""DiT block kernel for 8 trn2 NeuronCores.

Sharding: core c -> (batch b=c//2, query-token half h=c%2). Each core
computes the full block for its 512 query tokens (K/V compute for all
1024 tokens of its batch is replicated within the pair) -> zero
collectives. Activations are feature-major ([D on partitions, tokens on
free]); weights are used in natural [in, out] layout as matmul lhsT.

v3: weights are stored fp8 e4m3 (x64 host pre-scale so values sit in
e4m3's normal range; the 1/64 correction folds into each psum epilogue
or the residual gates) and feed mixed-dtype matmuls against f16
activations — half the weight DMA of the f16 baseline while keeping
DVE's 2x f16 throughput for the elementwise epilogues. x is loaded f16
(halves the head DMA and drops the LN1 stats copies). LN stats,
softmax normalization and residuals run in fp32/f16.
"""
import numpy as np

import concourse.bass as bass
import concourse.tile as tile
import concourse.mybir as mybir
from concourse.bass_utils import run_bass_kernel_spmd
from concourse.vector_clock import ScopedClock
from concourse.alu_op_type import AluOpType

dt = mybir.dt
AF = mybir.ActivationFunctionType

P = 128
B, NT, D, H = 4, 1024, 1024, 16
DH = D // H            # 64
DFF = 4 * D            # 4096
KC = D // P            # 8
LT = NT // 2           # 512 local query tokens
GATE = 0.1
EPS = 1e-5
EXP_SCALE = DH ** -0.5
EXP_BIAS = -3.0        # constant shift inside exp; cancels in softmax
WS = 64.0              # host pre-scale for fp8 weights
INV = 1.0 / WS


class SplitDrainTileContext(tile.TileContext):
    """Tail drain in this walrus build holds few sync waits; spill the
    rest onto chained SP nops (runs before the sem-clear barrier, so
    semantics are preserved)."""

    MAX_TAIL_WAITS = 1

    def _drain_and_barrier(self, tick_clock, wait_clock):
        drain_inst = self.nc.sync.drain()
        wait_clock.add_sem_waits(
            drain_inst.ins, ScopedClock({None: tick_clock.global_clock})
        )
        si = drain_inst.ins.sync_info
        waits = list(si.on_wait) if si else []
        if len(waits) > self.MAX_TAIL_WAITS:
            drain_inst.ins.sync_info = mybir.SyncInfo(
                on_wait=waits[: self.MAX_TAIL_WAITS],
                on_update=list(si.on_update) if si else [],
            )
            rest = waits[self.MAX_TAIL_WAITS:]
            for i in range(0, len(rest), self.MAX_TAIL_WAITS):
                nop = self.nc.sync.nop()
                nop.ins.sync_info = mybir.SyncInfo(
                    on_wait=rest[i : i + self.MAX_TAIL_WAITS], on_update=[]
                )
        self.nc.all_engine_barrier()
        assert self.sems is not None
        popped = self.nc._tile_sem_poison_stack.pop()
        assert popped is self._sem_poison
        self.nc.clear_and_free_semaphores(list(self.sems.allocated().values()))
        self.nc.all_engine_barrier()


def _legalize_waits(nc, max_waits=1):
    """This walrus build accepts at most one sync wait per instruction.
    Move surplus waits onto same-engine NoOps inserted just before the
    offending instruction (engine FIFO order preserves semantics)."""
    fix = 0
    for bb in nc.main_func.blocks:
        insts = list(bb.instructions)
        out = []
        for inst in insts:
            si = inst.sync_info
            waits = list(si.on_wait) if si else []
            if len(waits) > max_waits:
                keep = waits[-max_waits:]
                for w in waits[:-max_waits]:
                    nop = mybir.InstNoOp(name=f"I-wfix{fix}")
                    fix += 1
                    nop.engine = inst.engine
                    nop.sync_info = mybir.SyncInfo(on_wait=[w], on_update=[])
                    out.append(nop)
                inst.sync_info = mybir.SyncInfo(
                    on_wait=keep, on_update=list(si.on_update) if si else [])
            out.append(inst)
        if len(out) != len(insts):
            bb.instructions = out
    return fix


def _build(legalize=True):
    nc = bass.Bass(target_bir_lowering=False, debug=False,
                   dynamic_dma_scratch_size=2048)
    f32, f16, f8 = dt.float32, dt.float16, dt.float8e4

    xt = nc.dram_tensor("xt", [D, NT], f16, kind="ExternalInput")
    cond16 = nc.dram_tensor("cond16", [P, KC], f16, kind="ExternalInput")
    qkvw = nc.dram_tensor("qkvw", [D, 3 * D], f8, kind="ExternalInput")
    qkvbf = nc.dram_tensor("qkvbf", [P, 16], f32, kind="ExternalInput")
    bvrow = nc.dram_tensor("bvrow", [1, D], f16, kind="ExternalInput")
    projw = nc.dram_tensor("projw", [D, D], f8, kind="ExternalInput")
    projbrow = nc.dram_tensor("projbrow", [1, D], f16, kind="ExternalInput")
    fc1w = nc.dram_tensor("fc1w", [D, DFF], f8, kind="ExternalInput")
    fc1bf = nc.dram_tensor("fc1bf", [P, 32], f32, kind="ExternalInput")
    fc2w = nc.dram_tensor("fc2w", [D, DFF], f8, kind="ExternalInput")  # host-rearranged
    fc2brow = nc.dram_tensor("fc2brow", [1, D], f16, kind="ExternalInput")
    modw = nc.dram_tensor("modw", [12 * P, 4 * D], f8, kind="ExternalInput")  # host-rearranged
    modbf = nc.dram_tensor("modbf", [P, 6 * KC], f32, kind="ExternalInput")
    lnf = nc.dram_tensor("lnf", [P, 4 * KC], f32, kind="ExternalInput")
    outt = nc.dram_tensor("outt", [D, LT], f32, kind="ExternalOutput")

    with SplitDrainTileContext(nc) as tc:
        with tc.tile_pool(name="cp", bufs=1) as cp, \
             tc.tile_pool(name="ar", bufs=1) as ar, \
             tc.tile_pool(name="rot", bufs=4) as rot, \
             tc.tile_pool(name="psA", bufs=2, space="PSUM") as psA, \
             tc.tile_pool(name="psB", bufs=3, space="PSUM") as psB:

            def pp():    # [P, 512] f32 psum, 2 rotating banks
                return psA.tile([P, 512], f32, tag="pp", name="pp")

            def pbig():  # [P, 1024] f32 psum, 3 rotating 2-bank tiles
                return psB.tile([P, 1024], f32, tag="big", name="big")

            ones16 = cp.tile([P, P], f16, tag="ones16")
            nc.vector.memset(ones16[:], 1.0)
            onesrow = cp.tile([1, LT], f16, tag="onesrow")
            nc.vector.memset(onesrow[:], 1.0)
            expb = cp.tile([P, 1], f32, tag="expb")
            nc.vector.memset(expb[:], EXP_BIAS)

            # ---- resident small inputs ----
            condt = cp.tile([P, KC], f16, tag="condt")
            nc.sync.dma_start(condt[:], cond16[:])
            lnt = cp.tile([P, 4, KC], f32, tag="lnt")
            nc.sync.dma_start(lnt[:], lnf.rearrange("p (w c) -> p w c", c=KC))
            modbt = cp.tile([P, 6, KC], f32, tag="modbt")
            nc.sync.dma_start(modbt[:], modbf.rearrange("p (w c) -> p w c", c=KC))
            qkvbt = cp.tile([P, 16], f32, tag="qkvbt")
            nc.sync.dma_start(qkvbt[:], qkvbf[:])
            fc1bt = cp.tile([P, 32], f32, tag="fc1bt")
            nc.sync.dma_start(fc1bt[:], fc1bf[:])
            bvt = cp.tile([1, D], f16, tag="bvt")
            nc.sync.dma_start(bvt[:], bvrow[:])
            pbrow = cp.tile([1, D], f16, tag="pbrow")
            nc.sync.dma_start(pbrow[:], projbrow[:])
            f2brow = cp.tile([1, D], f16, tag="f2brow")
            nc.sync.dma_start(f2brow[:], fc2brow[:])

            # x, feature-major f16, 2 MiB (tag A4 later reused by h16)
            xf = ar.tile([P, KC, NT], f16, tag="A4")
            nc.sync.dma_start(xf[:], xt.rearrange("(c p) t -> p c t", p=P))

            # ---- S1: modulation matvecs (feature-major out) ----
            modv = cp.tile([P, 6, KC], f32, tag="modv")

            def mod_matvec(w):
                pm = pp()  # [P, 8] accum lives in a [P,512] slot
                for half in range(2):
                    mwt = ar.tile([P, KC, 512], f8, tag="Q1", bufs=3)
                    nc.sync.dma_start(
                        mwt[:],
                        modw[(w * 2 + half) * P:(w * 2 + half + 1) * P, :]
                        .rearrange("p (c m) -> p c m", m=512))
                    for mt in range(4):
                        mg = half * 4 + mt
                        for kc in range(KC):
                            nc.tensor.matmul(pm[:, mg:mg + 1],
                                             mwt[:, kc, mt * P:(mt + 1) * P],
                                             condt[:, kc:kc + 1],
                                             start=(kc == 0), stop=(kc == KC - 1))
                nc.vector.scalar_tensor_tensor(modv[:, w], pm[:, 0:KC], INV,
                                               modbt[:, w],
                                               AluOpType.mult, AluOpType.add)

            vecs = cp.tile([P, 6, KC], f32, tag="vecs")
            tgp = cp.tile([P, 2, KC], f32, tag="tgp")
            for w in range(2):
                mod_matvec(w)
            # scale1, shift1 (gate LN1 apply) from w0/w1 only
            nc.vector.tensor_scalar_add(tgp[:, 0], modv[:, 0], 1.0)
            nc.vector.tensor_tensor(vecs[:, 0], tgp[:, 0], lnt[:, 0], AluOpType.mult)
            nc.vector.tensor_tensor(vecs[:, 1], tgp[:, 0], lnt[:, 1], AluOpType.mult)
            nc.vector.tensor_tensor(vecs[:, 1], vecs[:, 1], modv[:, 1], AluOpType.add)
            # qkv weight block: put its DMA ahead of the remaining
            # modulation weights in the SP queue so qkv can start on time
            qkA = ar.tile([P, KC, 2 * D], f8, tag="W4")
            nc.sync.dma_start(qkA[:],
                              qkvw[:, 0:2 * D].rearrange("(c p) m -> p c m", p=P))
            mod_matvec(2)
            nc.scalar.activation(vecs[:, 2], modv[:, 2], AF.Tanh)
            nc.vector.tensor_scalar_mul(vecs[:, 2], vecs[:, 2], GATE * INV)

            def late_mod():
                for w in range(3, 6):
                    mod_matvec(w)
                nc.vector.tensor_scalar_add(tgp[:, 1], modv[:, 3], 1.0)
                nc.vector.tensor_tensor(vecs[:, 3], tgp[:, 1], lnt[:, 2],
                                        AluOpType.mult)
                nc.vector.tensor_tensor(vecs[:, 4], tgp[:, 1], lnt[:, 3],
                                        AluOpType.mult)
                nc.vector.tensor_tensor(vecs[:, 4], vecs[:, 4], modv[:, 4],
                                        AluOpType.add)
                nc.scalar.activation(vecs[:, 5], modv[:, 5], AF.Tanh)
                nc.vector.tensor_scalar_mul(vecs[:, 5], vecs[:, 5], GATE * INV)

            def r32(tag="R32"):
                return rot.tile([P, NT], f32, tag=tag, bufs=4, name="r32")

            def r16(tag="R16", bufs=3):
                return rot.tile([P, NT], f16, tag=tag, bufs=bufs, name="r16")

            def layernorm(src16, ntok, scale_col, shift_col, out16):
                """src16(j) -> f16 [P, ntok] AP used for stats and apply."""
                halves = ntok // 512
                pss = pbig()
                psq = pbig()
                for j in range(KC):
                    s16 = r16()
                    nc.vector.tensor_tensor(s16[:, 0:ntok], src16(j),
                                            src16(j), AluOpType.mult)
                    for nh in range(halves):
                        sl = slice(nh * 512, (nh + 1) * 512)
                        nc.tensor.matmul(pss[:, sl], ones16[:], src16(j)[:, sl],
                                         start=(j == 0), stop=(j == KC - 1),
                                         skip_group_check=True)
                        nc.tensor.matmul(psq[:, sl], ones16[:], s16[:, sl],
                                         start=(j == 0), stop=(j == KC - 1),
                                         skip_group_check=True)
                murep = r32()
                nc.vector.tensor_scalar_mul(murep[:, 0:ntok], pss[:, 0:ntok],
                                            1.0 / D)
                msq = r32()
                nc.vector.tensor_scalar(msq[:, 0:ntok], psq[:, 0:ntok],
                                        1.0 / D, EPS,
                                        AluOpType.mult, AluOpType.add)
                mu2 = r32()
                nc.vector.tensor_tensor(mu2[:, 0:ntok], murep[:, 0:ntok],
                                        murep[:, 0:ntok], AluOpType.mult)
                var = r32()
                nc.vector.tensor_tensor(var[:, 0:ntok], msq[:, 0:ntok],
                                        mu2[:, 0:ntok], AluOpType.subtract)
                rvar = r32()
                nc.vector.reciprocal(rvar[:, 0:ntok], var[:, 0:ntok])
                arep = r32()
                nc.scalar.activation(arep[:, 0:ntok], rvar[:, 0:ntok], AF.Sqrt)
                mur16 = r16("MU16", 2)
                nc.vector.tensor_copy(mur16[:, 0:ntok], murep[:, 0:ntok])
                ar16 = r16("MU16", 2)
                nc.vector.tensor_copy(ar16[:, 0:ntok], arep[:, 0:ntok])
                for j in range(KC):
                    t1 = r16()
                    nc.vector.tensor_tensor(t1[:, 0:ntok], src16(j),
                                            mur16[:, 0:ntok], AluOpType.subtract)
                    t2 = r16()
                    nc.vector.tensor_tensor(t2[:, 0:ntok], t1[:, 0:ntok],
                                            ar16[:, 0:ntok], AluOpType.mult)
                    nc.vector.tensor_scalar(out16[:, j], t2[:, 0:ntok],
                                            vecs[:, scale_col, j:j + 1],
                                            vecs[:, shift_col, j:j + 1],
                                            AluOpType.mult, AluOpType.add)

            late_mod()

            # ---- S2/S3: LN1 + modulate (all 1024 tokens) ----
            y16 = ar.tile([P, KC, NT], f16, tag="Y2", bufs=2)
            layernorm(lambda j: xf[:, j], NT, 0, 1, y16)

            # ---- S4: qkv ----
            q16 = ar.tile([P, KC, LT], f16, tag="Q1", bufs=3)
            k16 = ar.tile([P, KC, NT], f16, tag="K2")
            v16 = ar.tile([P, KC, D], f16, tag="V2")
            for mt in range(KC):  # q, local tokens
                pq = pp()
                for kc in range(KC):
                    nc.tensor.matmul(pq[:], qkA[:, kc, mt * P:(mt + 1) * P],
                                     y16[:, kc, 0:LT],
                                     start=(kc == 0), stop=(kc == KC - 1))
                nc.scalar.activation(q16[:, mt], pq[:], AF.Identity,
                                     bias=qkvbt[:, mt:mt + 1], scale=INV)
            for mt in range(KC):  # k, all tokens
                for nh in range(2):
                    pk = pp()
                    for kc in range(KC):
                        nc.tensor.matmul(
                            pk[:], qkA[:, kc, D + mt * P:D + (mt + 1) * P],
                            y16[:, kc, nh * 512:(nh + 1) * 512],
                            start=(kc == 0), stop=(kc == KC - 1))
                    nc.scalar.activation(k16[:, mt, nh * 512:(nh + 1) * 512],
                                         pk[:], AF.Identity,
                                         bias=qkvbt[:, 8 + mt:9 + mt],
                                         scale=INV)
            vW = ar.tile([P, KC, D], f8, tag="Y2", bufs=2)
            nc.sync.dma_start(vW[:],
                              qkvw[:, 2 * D:3 * D].rearrange("(c p) m -> p c m", p=P))
            pb = pbig()  # v bias replicated across partitions
            for nh in range(2):
                nc.tensor.matmul(pb[:, nh * 512:(nh + 1) * 512], ones16[0:1, :],
                                 bvt[:, nh * 512:(nh + 1) * 512],
                                 start=True, stop=True, skip_group_check=True)
            bvrep = r32()
            nc.vector.tensor_copy(bvrep[:], pb[:])
            for tt in range(KC):  # v rows = tokens (all)
                pv = pbig()
                for kc in range(KC):
                    for nh in range(2):
                        nc.tensor.matmul(
                            pv[:, nh * 512:(nh + 1) * 512],
                            y16[:, kc, tt * P:(tt + 1) * P],
                            vW[:, kc, nh * 512:(nh + 1) * 512],
                            start=(kc == 0), stop=(kc == KC - 1),
                            skip_group_check=True)
                nc.vector.scalar_tensor_tensor(v16[:, tt], pv[:], INV,
                                               bvrep[:],
                                               AluOpType.mult, AluOpType.add)

            # ---- S5: attention, head pair (2g, 2g+1) per feature tile g ----
            attn16 = ar.tile([P, KC, LT], f16, tag="AT")
            for g in range(KC):
                eg = ar.tile([P, KC, NT], f16, tag="Y2", bufs=2)
                for c in range(KC):
                    psc = pbig()
                    nc.tensor.matmul(psc[:, 0:512],
                                     k16[0:DH, g, c * P:(c + 1) * P],
                                     q16[0:DH, g, :], start=True, stop=True,
                                     skip_group_check=True)
                    nc.tensor.matmul(psc[:, 512:1024],
                                     k16[DH:P, g, c * P:(c + 1) * P],
                                     q16[DH:P, g, :], start=True, stop=True,
                                     skip_group_check=True)
                    nc.scalar.activation(eg[:, c], psc[:], AF.Exp,
                                         scale=EXP_SCALE, bias=expb[:])
                pse = pbig()
                for c in range(KC):
                    for nh in range(2):
                        nc.tensor.matmul(pse[:, nh * 512:(nh + 1) * 512],
                                         ones16[:],
                                         eg[:, c, nh * 512:(nh + 1) * 512],
                                         start=(c == 0), stop=(c == KC - 1),
                                         skip_group_check=True)
                recip = r32()
                nc.vector.reciprocal(recip[:], pse[:])
                pav = pp()
                for c in range(KC):
                    nc.tensor.matmul(pav[0:DH, :],
                                     v16[:, c, 2 * g * DH:(2 * g + 1) * DH],
                                     eg[:, c, 0:512],
                                     start=(c == 0), stop=(c == KC - 1),
                                     skip_group_check=True)
                    nc.tensor.matmul(pav[DH:P, :],
                                     v16[:, c, (2 * g + 1) * DH:(2 * g + 2) * DH],
                                     eg[:, c, 512:1024],
                                     start=(c == 0), stop=(c == KC - 1),
                                     skip_group_check=True, tile_position=(0, 64))
                nc.vector.tensor_tensor(attn16[0:DH, g], pav[0:DH, :],
                                        recip[0:DH, 0:512], AluOpType.mult)
                nc.vector.tensor_tensor(attn16[DH:P, g], pav[DH:P, :],
                                        recip[DH:P, 512:1024], AluOpType.mult)

            # ---- S6: proj + gated residual ----
            pw = ar.tile([P, KC, D], f8, tag="K2")
            nc.sync.dma_start(pw[:], projw.rearrange("(c p) m -> p c m", p=P))
            x2 = ar.tile([P, KC, LT], f32, tag="V2")
            for mt in range(KC):
                pj = pp()
                for kc in range(KC):
                    nc.tensor.matmul(pj[:], pw[:, kc, mt * P:(mt + 1) * P],
                                     attn16[:, kc, :],
                                     start=(kc == 0), stop=False)
                nc.tensor.matmul(pj[:], pbrow[:, mt * P:(mt + 1) * P],
                                 onesrow[:], start=False, stop=True)
                nc.vector.scalar_tensor_tensor(x2[:, mt], pj[:],
                                               vecs[:, 2, mt:mt + 1],
                                               xf[:, mt, 0:LT],
                                               AluOpType.mult, AluOpType.add)

            # ---- S7: LN2 + modulate (local tokens) ----
            z16 = ar.tile([P, KC, LT], f16, tag="Q1", bufs=3)
            c2 = ar.tile([P, KC, LT], f16, tag="C2")
            for j in range(KC):
                nc.scalar.activation(c2[:, j], x2[:, j], AF.Copy)
            layernorm(lambda j: c2[:, j], LT, 3, 4, z16)

            # ---- S8: fc1 + gelu ----
            h16 = ar.tile([P, 32, LT], f16, tag="A4")
            f1a = ar.tile([P, KC, 2 * D], f8, tag="W4")
            nc.sync.dma_start(f1a[:],
                              fc1w[:, 0:2 * D].rearrange("(c p) m -> p c m", p=P))
            f1b1 = ar.tile([P, KC, D], f8, tag="K2")
            nc.sync.dma_start(f1b1[:],
                              fc1w[:, 2 * D:3 * D].rearrange("(c p) m -> p c m", p=P))

            def fc1_block(wt, mg0, nmt):
                for mt in range(nmt):
                    mg = mg0 + mt
                    ph = pp()
                    for kc in range(KC):
                        nc.tensor.matmul(ph[:], wt[:, kc, mt * P:(mt + 1) * P],
                                         z16[:, kc, :],
                                         start=(kc == 0), stop=(kc == KC - 1))
                    nc.scalar.activation(h16[:, mg], ph[:], AF.Gelu,
                                         bias=fc1bt[:, mg:mg + 1], scale=INV)

            fc1_block(f1a, 0, 16)
            f1b2t = ar.tile([P, KC, D], f8, tag="W4")
            nc.sync.dma_start(f1b2t[:],
                              fc1w[:, 3 * D:4 * D].rearrange("(c p) m -> p c m", p=P))
            fc1_block(f1b1, 16, 8)
            fc1_block(f1b2t, 24, 8)

            # ---- S9: fc2 + gated residual + store ----
            for mt in range(KC):
                f2col = ar.tile([P, 32, P], f8, tag="Q1", bufs=3)
                nc.sync.dma_start(
                    f2col[:],
                    fc2w[mt * P:(mt + 1) * P, :]
                    .rearrange("p (c m) -> p c m", m=P))
                pz = pp()
                for kc in range(32):
                    nc.tensor.matmul(pz[:], f2col[:, kc, :], h16[:, kc, :],
                                     start=(kc == 0), stop=False)
                nc.tensor.matmul(pz[:], f2brow[:, mt * P:(mt + 1) * P],
                                 onesrow[:], start=False, stop=True)
                ot = rot.tile([P, LT], f32, tag="OT", bufs=2)
                nc.vector.scalar_tensor_tensor(ot[:], pz[:],
                                               vecs[:, 5, mt:mt + 1],
                                               x2[:, mt, :],
                                               AluOpType.mult, AluOpType.add)
                nc.sync.dma_start(outt[mt * P:(mt + 1) * P, :], ot[:])

    if legalize:
        _legalize_waits(nc)
    return nc


_NC_CACHE = {}


def _get_nc():
    if "nc" not in _NC_CACHE:
        _NC_CACHE["nc"] = _build()
    return _NC_CACHE["nc"]


def _feat(v, cols):
    """[D*]-vector -> feature-major [128, cols] (col j = chunk j)."""
    return np.ascontiguousarray(v.reshape(cols, P).T)


def make_in_maps(x, cond, g1_w, g1_b, b1_w, b1_b, a1_w, a1_b,
                 g2_w, g2_b, b2_w, b2_b, a2_w, a2_b,
                 ln1_g, ln1_b, ln2_g, ln2_b,
                 qkv_w, qkv_b, proj_w, proj_b,
                 fc1_w, fc1_b, fc2_w, fc2_b):
    f32 = np.float32
    f16 = np.float16
    f8 = dt.np(dt.float8e4)
    x = np.asarray(x, f32)
    cond = np.asarray(cond, f32)

    def w8(w):
        return (np.asarray(w, f32) * WS).astype(f8)

    shared = {
        "qkvw": w8(qkv_w),
        "qkvbf": np.hstack([_feat(np.asarray(qkv_b, f32)[0:D], KC),
                            _feat(np.asarray(qkv_b, f32)[D:2 * D], KC)]),
        "bvrow": np.asarray(qkv_b, f16)[None, 2 * D:3 * D],
        "projw": w8(proj_w),
        "projbrow": (np.asarray(proj_b, f32) * WS).astype(f16)[None, :],
        "fc1w": w8(fc1_w),
        "fc1bf": _feat(np.asarray(fc1_b, f32), 32),
        # [mt*128+p, kc*128+m] = fc2_w[kc*128+p, mt*128+m]: contiguous
        # per-mt loads of the feature-major lhsT tiles
        "fc2w": np.ascontiguousarray(
            w8(fc2_w).reshape(32, P, KC, P)
            .transpose(2, 1, 0, 3).reshape(D, DFF)),
        "fc2brow": (np.asarray(fc2_b, f32) * WS).astype(f16)[None, :],
        # row (w*2+half)*128+p, col kc*512+m = W_w[kc*128+p, half*512+m]:
        # contiguous loads of each feature-major half-block
        "modw": np.ascontiguousarray(
            np.hstack([(np.asarray(w, f32) * WS) for w in
                       (g1_w, b1_w, a1_w, g2_w, b2_w, a2_w)])
            .reshape(KC, P, 6, 2, 512).transpose(2, 3, 1, 0, 4)
            .reshape(12 * P, 4 * D)).astype(f8),
        "modbf": np.hstack([_feat(np.asarray(v, f32), KC) for v in
                            (g1_b, b1_b, a1_b, g2_b, b2_b, a2_b)]),
        "lnf": np.hstack([_feat(np.asarray(v, f32), KC) for v in
                          (ln1_g, ln1_b, ln2_g, ln2_b)]),
    }
    in_maps = []
    for c in range(8):
        b, h = c // 2, c % 2
        xb = x[b].T  # [D, NT]
        perm = np.concatenate([np.arange(h * LT, (h + 1) * LT),
                               np.arange((1 - h) * LT, (2 - h) * LT)])
        m = dict(shared)
        m["xt"] = np.ascontiguousarray(xb[:, perm]).astype(f16)
        m["cond16"] = _feat(cond[b], KC).astype(f16)
        in_maps.append(m)
    return in_maps


def kernel(**inputs):
    nc = _get_nc()
    in_maps = make_in_maps(**inputs)
    res = run_bass_kernel_spmd(nc, in_maps, list(range(8)))
    out = np.empty((B, NT, D), np.float32)
    for c in range(8):
        b, h = c // 2, c % 2
        out[b, h * LT:(h + 1) * LT, :] = res.results[c]["outt"].T
    return out
